# revision 19
# baseline (speedup 1.0000x reference)
"""Spatial attention block (GroupNorm + QKV 1x1 + full spatial attention +
out-proj + residual) on 8 Trainium2 NeuronCores.

Sharding: core = (batch b, spatial quarter j). Each core receives its batch
image rotated along the flattened spatial axis by -1024*j, so the SPMD
program always computes attention outputs for "the first 1024 query
positions" of its input. Attention is invariant to a joint rotation of the
K/V spatial axis, and GroupNorm stats are rotation-invariant, so the host
just concatenates the per-core [256, 1024] outputs.
"""

import sys

for _p in ("/opt/trn_rl_repo", "/root/.axon_site/_ro/trn_rl_repo"):
    if _p not in sys.path:
        sys.path.insert(0, _p)

import numpy as np

import concourse.bacc as bacc
import concourse.bass as bass
import concourse.tile as tile
from concourse import mybir
from concourse.bass_utils import run_bass_kernel_spmd

F32 = mybir.dt.float32
F32R = mybir.dt.float32r
BF16 = mybir.dt.bfloat16
AF = mybir.ActivationFunctionType


B, C, H, W = 2, 256, 64, 64
S = H * W              # 4096 spatial positions
NH = 4                 # heads
HD = C // NH           # 64 head dim
NQ = S // 4            # 1024 query positions per core
NCHUNK = S // 128      # 32 key chunks
EPS = 1e-5
SCALE = 1.0 / 16.0     # 1/sqrt(C)
VTW = 68               # per-head stride in the [V^T | ones] tile (64 V + 1 one + pad)


def _build_program():
    nc = bacc.Bacc(None)

    x_d = nc.declare_dram_parameter("x", [C, S], F32, isOutput=False)
    wqkvT_d = nc.declare_dram_parameter("wqkvT", [C, 3 * C], BF16, isOutput=False)
    woutT_d = nc.declare_dram_parameter("woutT", [NH, HD, C], BF16, isOutput=False)
    gnw_d = nc.declare_dram_parameter("gnw", [2, 128, 1], F32, isOutput=False)
    gnb_d = nc.declare_dram_parameter("gnb", [2, 128, 1], F32, isOutput=False)
    ob_d = nc.declare_dram_parameter("ob", [2, 128, 1], F32, isOutput=False)
    gsel_d = nc.declare_dram_parameter("gsel", [128, 8], F32R, isOutput=False)
    gselT_d = nc.declare_dram_parameter("gselT", [8, 128], F32R, isOutput=False)
    y_d = nc.declare_dram_parameter("y", [C, NQ], F32, isOutput=True)

    with tile.TileContext(nc) as tc, nc.allow_low_precision("fp32r matmul inputs"):
        _emit(nc, tc, x_d, wqkvT_d, woutT_d, gnw_d, gnb_d, ob_d, gsel_d, gselT_d, y_d)
    nc.finalize()
    return nc


def _emit(nc, tc, x_d, wqkvT_d, woutT_d, gnw_d, gnb_d, ob_d, gsel_d, gselT_d, y_d):
    from contextlib import ExitStack

    ctx = ExitStack()
    with ctx:
        persist = ctx.enter_context(tc.tile_pool(name="persist", bufs=1))
        pp = ctx.enter_context(tc.tile_pool(name="pp", bufs=2, space="PSUM"))
        po = ctx.enter_context(tc.tile_pool(name="po", bufs=2, space="PSUM"))

        # ---- persistent SBUF tiles -------------------------------------
        x_sb = [persist.tile([128, S], F32, tag=f"x{t}", name=f"x{t}") for t in range(2)]
        k_sb = [persist.tile([128, S], BF16, tag=f"k{t}", name=f"k{t}") for t in range(2)]
        q_sb = [persist.tile([128, NQ], BF16, tag=f"q{t}", name=f"q{t}") for t in range(2)]
        vt_sb = [persist.tile([128, NH * VTW], BF16, tag=f"vt{c}", name=f"vt{c}") for c in range(NCHUNK)]
        attn_sb = [persist.tile([64, NQ], BF16, tag=f"at{h}", name=f"at{h}") for h in range(NH)]
        wq_sb = [persist.tile([128, 3 * C], BF16, tag=f"wq{t}", name=f"wq{t}") for t in range(2)]
        wo_sb = [persist.tile([HD, C], BF16, tag=f"wo{ct}", name=f"wo{ct}") for ct in range(NH)]
        gnw_sb = [persist.tile([128, 1], F32, tag=f"gw{t}", name=f"gw{t}") for t in range(2)]
        gnb_sb = [persist.tile([128, 1], F32, tag=f"gb{t}", name=f"gb{t}") for t in range(2)]
        ob_sb = [persist.tile([128, 1], F32, tag=f"obias{t}", name=f"obias{t}") for t in range(2)]
        gsel_sb = persist.tile([128, 8], F32R, tag="gsel")
        gselT_sb = persist.tile([8, 128], F32R, tag="gselT")
        oacc_sb = [
            persist.tile([128, NQ], F32, tag=f"oacc{t}", name=f"oacc{t}")
            for t in range(2)
        ]
        eps_sb = persist.tile([128, 1], F32, tag="eps")
        nc.vector.memset(eps_sb, EPS)
        ones64_sb = persist.tile([1, 64], F32R, tag="ones64")
        nc.scalar.activation(
            out=ones64_sb, in_=x_sb[0][0:1, 0:64], func=AF.Identity, scale=0.0, bias=1.0
        )

        for t in range(2):
            for xc in range(4):
                nc.sync.dma_start(
                    out=x_sb[t][:, 1024 * xc : 1024 * (xc + 1)],
                    in_=x_d[128 * t : 128 * (t + 1), 1024 * xc : 1024 * (xc + 1)],
                )
            nc.sync.dma_start(out=wq_sb[t], in_=wqkvT_d[128 * t : 128 * (t + 1), :])
            nc.sync.dma_start(out=gnw_sb[t], in_=gnw_d[t])
            nc.sync.dma_start(out=gnb_sb[t], in_=gnb_d[t])
            nc.sync.dma_start(out=ob_sb[t], in_=ob_d[t])
        for ct in range(NH):
            nc.sync.dma_start(out=wo_sb[ct], in_=woutT_d[ct])
        nc.sync.dma_start(out=gsel_sb, in_=gsel_d[:])
        nc.sync.dma_start(out=gselT_sb, in_=gselT_d[:])

        # ones columns of the [V^T | ones] tiles
        for c in range(NCHUNK):
            ones_cols = vt_sb[c].rearrange("p (h x) -> p h x", h=NH)[:, :, HD : HD + 1]
            nc.vector.memset(ones_cols, 1.0)

        # ---- GroupNorm -------------------------------------------------
        # per-channel stats via bn_stats (free-dim), then combine the 16
        # channels of each group across partitions with small PE matmuls.
        with tc.tile_pool(name="gnpool", bufs=1) as gnp, tc.tile_pool(
            name="xn", bufs=1
        ) as xnp:
            xn_sb = [xnp.tile([128, S], BF16, tag=f"xn{t}", name=f"xn{t}") for t in range(2)]
            s_t = []
            b_t = []
            for t in range(2):
                nsub = S // 512
                st6 = gnp.tile([128, nsub, 6], F32, tag=f"st6_{t}")
                for i in range(nsub):
                    nc.vector.bn_stats(
                        out=st6[:, i, :], in_=x_sb[t][:, 512 * i : 512 * (i + 1)]
                    )
                mv = gnp.tile([128, 2], F32, tag=f"mv{t}")
                nc.vector.bn_aggr(out=mv, in_=st6)
                # stats2 = [mean, var + mean^2]  (per channel)
                stats2 = gnp.tile([128, 2], F32R, tag=f"s2_{t}")
                nc.vector.tensor_copy(out=stats2[:, 0:1], in_=mv[:, 0:1])
                nc.vector.tensor_tensor(
                    out=stats2[:, 1:2],
                    in0=mv[:, 0:1],
                    in1=mv[:, 0:1],
                    op=mybir.AluOpType.mult,
                )
                nc.vector.tensor_tensor(
                    out=stats2[:, 1:2],
                    in0=stats2[:, 1:2],
                    in1=mv[:, 1:2],
                    op=mybir.AluOpType.add,
                )
                # group sums: [8, 2] = gsel.T @ stats2, then /16
                pg = pp.tile([8, 2], F32, tag="pp")
                nc.tensor.matmul(pg, (gsel_sb), (stats2), start=True, stop=True)
                g2 = gnp.tile([8, 2], F32, tag=f"g2_{t}")
                nc.scalar.activation(out=g2, in_=pg, func=AF.Copy, scale=1.0 / 16.0)
                # var_g = m2_g - mu_g^2 ; rstd = 1/sqrt(var+eps)
                mr = gnp.tile([8, 2], F32R, tag=f"mr{t}")
                nc.vector.tensor_copy(out=mr[:, 0:1], in_=g2[:, 0:1])
                vg = gnp.tile([8, 1], F32, tag=f"vg{t}")
                nc.vector.tensor_tensor(
                    out=vg, in0=g2[:, 0:1], in1=g2[:, 0:1], op=mybir.AluOpType.mult
                )
                nc.vector.tensor_tensor(
                    out=vg, in0=g2[:, 1:2], in1=vg, op=mybir.AluOpType.subtract
                )
                nc.scalar.activation(out=vg, in_=vg, func=AF.Ln, bias=eps_sb[0:8])
                nc.scalar.activation(out=mr[:, 1:2], in_=vg, func=AF.Exp, scale=-0.5)
                # broadcast (mu, rstd) to the 16 channels of each group
                pb = pp.tile([128, 2], F32, tag="pp")
                nc.tensor.matmul(pb, (gselT_sb), (mr), start=True, stop=True)
                # scale = gnw * rstd ; bias = gnb - mu * scale
                sc = gnp.tile([128, 1], F32, tag=f"sc{t}")
                bi = gnp.tile([128, 1], F32, tag=f"bi{t}")
                nc.vector.tensor_tensor(
                    out=sc, in0=gnw_sb[t], in1=pb[:, 1:2], op=mybir.AluOpType.mult
                )
                nc.vector.tensor_tensor(
                    out=bi, in0=pb[:, 0:1], in1=sc, op=mybir.AluOpType.mult
                )
                nc.vector.tensor_tensor(
                    out=bi, in0=gnb_sb[t], in1=bi, op=mybir.AluOpType.subtract
                )
                s_t.append(sc)
                b_t.append(bi)
            for t in range(2):
                nc.vector.tensor_scalar(
                    out=xn_sb[t],
                    in0=x_sb[t],
                    scalar1=s_t[t],
                    scalar2=b_t[t],
                    op0=mybir.AluOpType.mult,
                    op1=mybir.AluOpType.add,
                )

            # ---- projections (inside xn pool scope) --------------------
            # K: [256 kch, S];  kch tile t holds heads 2t, 2t+1
            for t in range(2):
                for sb in range(S // 512):
                    ps = pp.tile([128, 512], F32, tag="pp")
                    for kc in range(2):
                        nc.tensor.matmul(
                            ps,
                            (wq_sb[kc][:, C + 128 * t : C + 128 * (t + 1)]),
                            (xn_sb[kc][:, 512 * sb : 512 * (sb + 1)]),
                            start=(kc == 0),
                            stop=(kc == 1),
                        )
                    nc.scalar.activation(
                        out=k_sb[t][:, 512 * sb : 512 * (sb + 1)], in_=ps, func=AF.Copy
                    )
            # Q: first NQ positions only
            for t in range(2):
                for sb in range(NQ // 512):
                    ps = pp.tile([128, 512], F32, tag="pp")
                    for kc in range(2):
                        nc.tensor.matmul(
                            ps,
                            (wq_sb[kc][:, 128 * t : 128 * (t + 1)]),
                            (xn_sb[kc][:, 512 * sb : 512 * (sb + 1)]),
                            start=(kc == 0),
                            stop=(kc == 1),
                        )
                    nc.scalar.activation(
                        out=q_sb[t][:, 512 * sb : 512 * (sb + 1)], in_=ps, func=AF.Copy
                    )
            # V^T: [S, 256] in chunks of 128 rows, written into the strided
            # per-head [V^T | 1] layout.
            for c in range(NCHUNK):
                ps = pp.tile([128, C], F32, tag="pp")
                for kc in range(2):
                    nc.tensor.matmul(
                        ps,
                        (xn_sb[kc][:, 128 * c : 128 * (c + 1)]),
                        (wq_sb[kc][:, 2 * C : 3 * C]),
                        start=(kc == 0),
                        stop=(kc == 1),
                    )
                vdst = vt_sb[c].rearrange("p (h x) -> p h x", h=NH)[:, :, 0:HD]
                nc.vector.tensor_copy(out=vdst, in_=ps.rearrange("p (h x) -> p h x", h=NH))

        # ---- attention -------------------------------------------------
        with tc.tile_pool(name="epool", bufs=3) as ep, tc.tile_pool(
            name="npool", bufs=2
        ) as np_pool, tc.tile_pool(name="dpool", bufs=2, space="DRAM") as dp:
            def head_epilogue(h, po_h):
                # evacuate the accumulator to SBUF right away so the PSUM
                # slot frees for the next head's AV matmuls
                oh = np_pool.tile([65, NQ], F32, tag="oh", name=f"oh{h}")
                nc.scalar.activation(out=oh, in_=po_h, func=AF.Copy)
                # 1/denominator, then broadcast across 64 partitions via a
                # K=1 ones matmul on the PE
                rsum = np_pool.tile([1, NQ], F32, tag="rs", name=f"rs{h}")
                nc.vector.reciprocal(out=rsum, in_=oh[64:65, :])
                rs_d = dp.tile([1, NQ], F32, tag="rsd", name=f"rsd{h}")
                nc.sync.dma_start(out=rs_d, in_=rsum)
                bc = np_pool.tile([64, NQ], F32, tag="bc", name=f"bc{h}")
                bcast_src = bass.AP(
                    tensor=rs_d.tensor, offset=rs_d.offset, ap=[[0, 64]] + rs_d.ap[1:]
                )
                nc.sync.dma_start(out=bc, in_=bcast_src)
                nc.vector.tensor_tensor(
                    out=attn_sb[h], in0=oh[0:64, :], in1=bc, op=mybir.AluOpType.mult
                )
                # streaming out-projection: fold this head's contribution in
                for oct_ in range(2):
                    pf = pp.tile([128, NQ], F32, tag="pp", name=f"pf{h}_{oct_}")
                    for nb in range(NQ // 512):
                        nc.tensor.matmul(
                            pf[:, 512 * nb : 512 * (nb + 1)],
                            (wo_sb[h][:, 128 * oct_ : 128 * (oct_ + 1)]),
                            attn_sb[h][:, 512 * nb : 512 * (nb + 1)],
                            start=True,
                            stop=True,
                        )
                    if h == 0:
                        nc.vector.tensor_copy(out=oacc_sb[oct_], in_=pf)
                    else:
                        nc.vector.tensor_tensor(
                            out=oacc_sb[oct_],
                            in0=oacc_sb[oct_],
                            in1=pf,
                            op=mybir.AluOpType.add,
                        )

            # heads are processed in pairs; the pair's QK matmuls target
            # disjoint PE row groups (rows 0-63 / 64-127) and run
            # concurrently in the array.
            for pair in range(NH // 2):
                po_pair = [
                    po.tile([65, NQ], F32, tag="po", name=f"po{pair}_{i}")
                    for i in range(2)
                ]
                for c in range(NCHUNK):
                    ps_pair = [
                        pp.tile([128, NQ], F32, tag="pp", name=f"ps{pair}_{c}_{i}")
                        for i in range(2)
                    ]
                    for nb in range(NQ // 512):
                        for i in range(2):
                            row = i * 64
                            nc.tensor.matmul(
                                ps_pair[i][:, 512 * nb : 512 * (nb + 1)],
                                (k_sb[pair][row : row + 64, 128 * c : 128 * (c + 1)]),
                                (q_sb[pair][row : row + 64, 512 * nb : 512 * (nb + 1)]),
                                start=True,
                                stop=True,
                            )
                    for i in range(2):
                        h = 2 * pair + i
                        e_t = ep.tile([128, NQ], BF16, tag="e", name=f"e{pair}_{c}_{i}")
                        nc.scalar.activation(
                            out=e_t, in_=ps_pair[i], func=AF.Exp, scale=SCALE
                        )
                        for nb in range(NQ // 512):
                            nc.tensor.matmul(
                                po_pair[i][:, 512 * nb : 512 * (nb + 1)],
                                (vt_sb[c][:, VTW * h : VTW * h + HD + 1]),
                                e_t[:, 512 * nb : 512 * (nb + 1)],
                                start=(c == 0),
                                stop=(c == NCHUNK - 1),
                            )
                for i in range(2):
                    head_epilogue(2 * pair + i, po_pair[i])

        # ---- bias + residual + store -----------------------------------
        with tc.tile_pool(name="fpool", bufs=2) as fp:
            for oct_ in range(2):
                fo = fp.tile([128, NQ], F32, tag="fo")
                nc.scalar.activation(
                    out=fo, in_=oacc_sb[oct_], func=AF.Identity, bias=ob_sb[oct_]
                )
                nc.vector.tensor_tensor(
                    out=fo, in0=fo, in1=x_sb[oct_][:, 0:NQ], op=mybir.AluOpType.add
                )
                nc.sync.dma_start(out=y_d[128 * oct_ : 128 * (oct_ + 1), :], in_=fo)


_PROGRAM = None


def _get_program():
    global _PROGRAM
    if _PROGRAM is None:
        _PROGRAM = _build_program()
    return _PROGRAM


def _prep_inputs(input, gn_weight, gn_bias, qkv_weight, out_weight, out_bias):
    input = np.asarray(input, dtype=np.float32).reshape(B, C, S)
    gn_weight = np.asarray(gn_weight, dtype=np.float32)
    gn_bias = np.asarray(gn_bias, dtype=np.float32)
    qkv_weight = np.asarray(qkv_weight, dtype=np.float32)
    out_weight = np.asarray(out_weight, dtype=np.float32)
    out_bias = np.asarray(out_bias, dtype=np.float32)

    # reference splits qkv head-major: rows 192h..192h+192 = [q|k|v] of head h.
    # device layout wants cols [Q heads 0..3 | K heads 0..3 | V heads 0..3].
    perm = np.concatenate(
        [
            np.arange(192 * h + 64 * part, 192 * h + 64 * (part + 1))
            for part in range(3)
            for h in range(NH)
        ]
    )
    import ml_dtypes

    wqkvT = np.ascontiguousarray(qkv_weight.T[:, perm]).astype(ml_dtypes.bfloat16)
    # reorder: per head h, q rows h*64..h*64+64 map to wqkvT cols as-is
    woutT = np.ascontiguousarray(out_weight.T.reshape(NH, HD, C)).astype(
        ml_dtypes.bfloat16
    )
    gnw = np.ascontiguousarray(gn_weight.reshape(2, 128, 1))
    gnb = np.ascontiguousarray(gn_bias.reshape(2, 128, 1))
    ob = np.ascontiguousarray(out_bias.reshape(2, 128, 1))
    gsel = np.zeros((128, 8), np.float32)
    for p in range(128):
        gsel[p, p // 16] = 1.0
    gselT = np.ascontiguousarray(gsel.T)

    in_maps = []
    for core in range(8):
        b, j = core // 4, core % 4
        xrot = np.roll(input[b], -NQ * j, axis=1)
        in_maps.append(
            {
                "x": np.ascontiguousarray(xrot),
                "wqkvT": wqkvT,
                "woutT": woutT,
                "gnw": gnw,
                "gnb": gnb,
                "ob": ob,
                "gsel": gsel,
                "gselT": gselT,
            }
        )
    return in_maps


def kernel(input, gn_weight, gn_bias, qkv_weight, out_weight, out_bias, _trace=False):
    nc = _get_program()
    in_maps = _prep_inputs(
        input, gn_weight, gn_bias, qkv_weight, out_weight, out_bias
    )
    kw = {}
    if _trace:
        kw = {"trace": True, "tmpdir": "/tmp/attn_trace"}
    res = run_bass_kernel_spmd(nc, in_maps, list(range(8)), **kw)
    out = np.empty((B, C, S), np.float32)
    for core in range(8):
        b, j = core // 4, core % 4
        out[b, :, NQ * j : NQ * (j + 1)] = res.results[core]["y"]
    out = out.reshape(B, C, H, W)
    if _trace:
        return out, res
    return out


# revision 20
# speedup vs baseline: 1.4269x; 1.4269x over previous
"""Spatial attention block (GroupNorm + QKV 1x1 + full spatial attention +
out-proj + residual) on 8 Trainium2 NeuronCores.

Sharding: core = (batch b, spatial quarter j). Each core receives its batch
image rotated along the flattened spatial axis by -1024*j, so the SPMD
program always computes attention outputs for "the first 1024 query
positions" of its input. Attention is invariant to a joint rotation of the
K/V spatial axis, and GroupNorm stats are rotation-invariant, so the host
just concatenates the per-core [256, 1024] outputs.
"""

import sys

for _p in ("/opt/trn_rl_repo", "/root/.axon_site/_ro/trn_rl_repo"):
    if _p not in sys.path:
        sys.path.insert(0, _p)

import numpy as np

import concourse.bacc as bacc
import concourse.bass as bass
import concourse.tile as tile
from concourse import mybir
from concourse.bass_utils import run_bass_kernel_spmd

F32 = mybir.dt.float32
F32R = mybir.dt.float32r
BF16 = mybir.dt.bfloat16
AF = mybir.ActivationFunctionType


B, C, H, W = 2, 256, 64, 64
S = H * W              # 4096 spatial positions
NH = 4                 # heads
HD = C // NH           # 64 head dim
NQ = S // 4            # 1024 query positions per core
NCHUNK = S // 128      # 32 key chunks
EPS = 1e-5
SCALE = 1.0 / 16.0     # 1/sqrt(C)
VTW = 68               # per-head stride in the [V^T | ones] tile (64 V + 1 one + pad)


def _build_program():
    nc = bacc.Bacc(None)

    x_d = nc.declare_dram_parameter("x", [C, S], F32, isOutput=False)
    wqkvT_d = nc.declare_dram_parameter("wqkvT", [C, 3 * C], BF16, isOutput=False)
    woutT_d = nc.declare_dram_parameter("woutT", [NH, HD, C], BF16, isOutput=False)
    gnw_d = nc.declare_dram_parameter("gnw", [2, 128, 1], F32, isOutput=False)
    gnb_d = nc.declare_dram_parameter("gnb", [2, 128, 1], F32, isOutput=False)
    ob_d = nc.declare_dram_parameter("ob", [2, 128, 1], F32, isOutput=False)
    gsel_d = nc.declare_dram_parameter("gsel", [128, 8], F32R, isOutput=False)
    gselT_d = nc.declare_dram_parameter("gselT", [8, 128], F32R, isOutput=False)
    y_d = nc.declare_dram_parameter("y", [C, NQ], F32, isOutput=True)

    with tile.TileContext(nc) as tc, nc.allow_low_precision("fp32r matmul inputs"):
        _emit(nc, tc, x_d, wqkvT_d, woutT_d, gnw_d, gnb_d, ob_d, gsel_d, gselT_d, y_d)
    nc.finalize()
    return nc


def _emit(nc, tc, x_d, wqkvT_d, woutT_d, gnw_d, gnb_d, ob_d, gsel_d, gselT_d, y_d):
    from contextlib import ExitStack

    ctx = ExitStack()
    with ctx:
        persist = ctx.enter_context(tc.tile_pool(name="persist", bufs=1))
        pp = ctx.enter_context(tc.tile_pool(name="pp", bufs=2, space="PSUM"))
        po = ctx.enter_context(tc.tile_pool(name="po", bufs=2, space="PSUM"))

        # ---- persistent SBUF tiles -------------------------------------
        x_sb = [persist.tile([128, S], F32, tag=f"x{t}", name=f"x{t}") for t in range(2)]
        k_sb = [persist.tile([128, S], BF16, tag=f"k{t}", name=f"k{t}") for t in range(2)]
        q_sb = [persist.tile([128, NQ], BF16, tag=f"q{t}", name=f"q{t}") for t in range(2)]
        vt_sb = [persist.tile([128, NH * VTW], BF16, tag=f"vt{c}", name=f"vt{c}") for c in range(NCHUNK)]
        attn_sb = [persist.tile([64, NQ], BF16, tag=f"at{h}", name=f"at{h}") for h in range(NH)]
        wq_sb = [persist.tile([128, 3 * C], BF16, tag=f"wq{t}", name=f"wq{t}") for t in range(2)]
        wo_sb = [persist.tile([HD, C], BF16, tag=f"wo{ct}", name=f"wo{ct}") for ct in range(NH)]
        gnw_sb = [persist.tile([128, 1], F32, tag=f"gw{t}", name=f"gw{t}") for t in range(2)]
        gnb_sb = [persist.tile([128, 1], F32, tag=f"gb{t}", name=f"gb{t}") for t in range(2)]
        ob_sb = [persist.tile([128, 1], F32, tag=f"obias{t}", name=f"obias{t}") for t in range(2)]
        gsel_sb = persist.tile([128, 8], F32R, tag="gsel")
        gselT_sb = persist.tile([8, 128], F32R, tag="gselT")
        oacc_sb = [
            persist.tile([128, NQ], F32, tag=f"oacc{t}", name=f"oacc{t}")
            for t in range(2)
        ]
        eps_sb = persist.tile([128, 1], F32, tag="eps")
        nc.vector.memset(eps_sb, EPS)
        ones64_sb = persist.tile([1, 64], F32R, tag="ones64")
        nc.scalar.activation(
            out=ones64_sb, in_=x_sb[0][0:1, 0:64], func=AF.Identity, scale=0.0, bias=1.0
        )

        for t in range(2):
            for xc in range(4):
                nc.sync.dma_start(
                    out=x_sb[t][:, 1024 * xc : 1024 * (xc + 1)],
                    in_=x_d[128 * t : 128 * (t + 1), 1024 * xc : 1024 * (xc + 1)],
                )
            nc.sync.dma_start(out=wq_sb[t], in_=wqkvT_d[128 * t : 128 * (t + 1), :])
            nc.sync.dma_start(out=gnw_sb[t], in_=gnw_d[t])
            nc.sync.dma_start(out=gnb_sb[t], in_=gnb_d[t])
            nc.sync.dma_start(out=ob_sb[t], in_=ob_d[t])
        for ct in range(NH):
            nc.sync.dma_start(out=wo_sb[ct], in_=woutT_d[ct])
        nc.sync.dma_start(out=gsel_sb, in_=gsel_d[:])
        nc.sync.dma_start(out=gselT_sb, in_=gselT_d[:])

        # ones columns of the [V^T | ones] tiles
        for c in range(NCHUNK):
            ones_cols = vt_sb[c].rearrange("p (h x) -> p h x", h=NH)[:, :, HD : HD + 1]
            nc.vector.memset(ones_cols, 1.0)

        # ---- GroupNorm -------------------------------------------------
        # per-channel stats via bn_stats (free-dim), then combine the 16
        # channels of each group across partitions with small PE matmuls.
        with tc.tile_pool(name="gnpool", bufs=1) as gnp, tc.tile_pool(
            name="xn", bufs=1
        ) as xnp:
            xn_sb = [xnp.tile([128, S], BF16, tag=f"xn{t}", name=f"xn{t}") for t in range(2)]
            s_t = []
            b_t = []
            for t in range(2):
                nsub = S // 512
                st6 = gnp.tile([128, nsub, 6], F32, tag=f"st6_{t}")
                for i in range(nsub):
                    nc.vector.bn_stats(
                        out=st6[:, i, :], in_=x_sb[t][:, 512 * i : 512 * (i + 1)]
                    )
                mv = gnp.tile([128, 2], F32, tag=f"mv{t}")
                nc.vector.bn_aggr(out=mv, in_=st6)
                # stats2 = [mean, var + mean^2]  (per channel)
                stats2 = gnp.tile([128, 2], F32R, tag=f"s2_{t}")
                nc.vector.tensor_copy(out=stats2[:, 0:1], in_=mv[:, 0:1])
                nc.vector.tensor_tensor(
                    out=stats2[:, 1:2],
                    in0=mv[:, 0:1],
                    in1=mv[:, 0:1],
                    op=mybir.AluOpType.mult,
                )
                nc.vector.tensor_tensor(
                    out=stats2[:, 1:2],
                    in0=stats2[:, 1:2],
                    in1=mv[:, 1:2],
                    op=mybir.AluOpType.add,
                )
                # group sums: [8, 2] = gsel.T @ stats2, then /16
                pg = pp.tile([8, 2], F32, tag="pp")
                nc.tensor.matmul(pg, (gsel_sb), (stats2), start=True, stop=True)
                g2 = gnp.tile([8, 2], F32, tag=f"g2_{t}")
                nc.scalar.activation(out=g2, in_=pg, func=AF.Copy, scale=1.0 / 16.0)
                # var_g = m2_g - mu_g^2 ; rstd = 1/sqrt(var+eps)
                mr = gnp.tile([8, 2], F32R, tag=f"mr{t}")
                nc.vector.tensor_copy(out=mr[:, 0:1], in_=g2[:, 0:1])
                vg = gnp.tile([8, 1], F32, tag=f"vg{t}")
                nc.vector.tensor_tensor(
                    out=vg, in0=g2[:, 0:1], in1=g2[:, 0:1], op=mybir.AluOpType.mult
                )
                nc.vector.tensor_tensor(
                    out=vg, in0=g2[:, 1:2], in1=vg, op=mybir.AluOpType.subtract
                )
                nc.scalar.activation(out=vg, in_=vg, func=AF.Ln, bias=eps_sb[0:8])
                nc.scalar.activation(out=mr[:, 1:2], in_=vg, func=AF.Exp, scale=-0.5)
                # broadcast (mu, rstd) to the 16 channels of each group
                pb = pp.tile([128, 2], F32, tag="pp")
                nc.tensor.matmul(pb, (gselT_sb), (mr), start=True, stop=True)
                # scale = gnw * rstd ; bias = gnb - mu * scale
                sc = gnp.tile([128, 1], F32, tag=f"sc{t}")
                bi = gnp.tile([128, 1], F32, tag=f"bi{t}")
                nc.vector.tensor_tensor(
                    out=sc, in0=gnw_sb[t], in1=pb[:, 1:2], op=mybir.AluOpType.mult
                )
                nc.vector.tensor_tensor(
                    out=bi, in0=pb[:, 0:1], in1=sc, op=mybir.AluOpType.mult
                )
                nc.vector.tensor_tensor(
                    out=bi, in0=gnb_sb[t], in1=bi, op=mybir.AluOpType.subtract
                )
                s_t.append(sc)
                b_t.append(bi)
            for t in range(2):
                nc.vector.tensor_scalar(
                    out=xn_sb[t],
                    in0=x_sb[t],
                    scalar1=s_t[t],
                    scalar2=b_t[t],
                    op0=mybir.AluOpType.mult,
                    op1=mybir.AluOpType.add,
                )

            # ---- projections (inside xn pool scope) --------------------
            # K: [256 kch, S];  kch tile t holds heads 2t, 2t+1
            for t in range(2):
                for sb in range(S // 512):
                    ps = pp.tile([128, 512], F32, tag="pp")
                    for kc in range(2):
                        nc.tensor.matmul(
                            ps,
                            (wq_sb[kc][:, C + 128 * t : C + 128 * (t + 1)]),
                            (xn_sb[kc][:, 512 * sb : 512 * (sb + 1)]),
                            start=(kc == 0),
                            stop=(kc == 1),
                        )
                    nc.scalar.activation(
                        out=k_sb[t][:, 512 * sb : 512 * (sb + 1)], in_=ps, func=AF.Copy
                    )
            # Q: first NQ positions only
            for t in range(2):
                for sb in range(NQ // 512):
                    ps = pp.tile([128, 512], F32, tag="pp")
                    for kc in range(2):
                        nc.tensor.matmul(
                            ps,
                            (wq_sb[kc][:, 128 * t : 128 * (t + 1)]),
                            (xn_sb[kc][:, 512 * sb : 512 * (sb + 1)]),
                            start=(kc == 0),
                            stop=(kc == 1),
                        )
                    nc.scalar.activation(
                        out=q_sb[t][:, 512 * sb : 512 * (sb + 1)], in_=ps, func=AF.Copy
                    )
            # V^T: [S, 256] in chunks of 128 rows, written into the strided
            # per-head [V^T | 1] layout.
            for c in range(NCHUNK):
                ps = pp.tile([128, C], F32, tag="pp")
                for kc in range(2):
                    nc.tensor.matmul(
                        ps,
                        (xn_sb[kc][:, 128 * c : 128 * (c + 1)]),
                        (wq_sb[kc][:, 2 * C : 3 * C]),
                        start=(kc == 0),
                        stop=(kc == 1),
                    )
                vdst = vt_sb[c].rearrange("p (h x) -> p h x", h=NH)[:, :, 0:HD]
                nc.vector.tensor_copy(out=vdst, in_=ps.rearrange("p (h x) -> p h x", h=NH))

        # ---- attention -------------------------------------------------
        with tc.tile_pool(name="epool", bufs=3) as ep, tc.tile_pool(
            name="npool", bufs=2
        ) as np_pool, tc.tile_pool(name="dpool", bufs=2, space="DRAM") as dp:
            def head_epilogue(h, po_h):
                # evacuate the accumulator to SBUF right away so the PSUM
                # slot frees for the next pair's AV matmuls; reciprocal of
                # the denominator row runs on DVE off the critical path.
                oh = np_pool.tile([65, NQ], F32, tag=f"oh{h}", name=f"oh{h}")
                nc.scalar.activation(out=oh, in_=po_h, func=AF.Copy)
                rsum = np_pool.tile([1, NQ], F32R, tag=f"rs{h}", name=f"rs{h}")
                nc.vector.reciprocal(out=rsum, in_=oh[64:65, :])
                oh_l.append(oh)
                rs_l.append(rsum)

            # heads are processed in pairs; the pair's QK matmuls target
            # disjoint PE row groups (rows 0-63 / 64-127) and run
            # concurrently in the array.
            oh_l = []
            rs_l = []
            for pair in range(NH // 2):
                po_pair = [
                    po.tile([65, NQ], F32, tag="po", name=f"po{pair}_{i}")
                    for i in range(2)
                ]
                for c in range(NCHUNK):
                    ps_pair = [
                        pp.tile([128, NQ], F32, tag="pp", name=f"ps{pair}_{c}_{i}")
                        for i in range(2)
                    ]
                    for nb in range(NQ // 512):
                        for i in range(2):
                            row = i * 64
                            nc.tensor.matmul(
                                ps_pair[i][:, 512 * nb : 512 * (nb + 1)],
                                (k_sb[pair][row : row + 64, 128 * c : 128 * (c + 1)]),
                                (q_sb[pair][row : row + 64, 512 * nb : 512 * (nb + 1)]),
                                start=True,
                                stop=True,
                            )
                    for i in range(2):
                        h = 2 * pair + i
                        e_t = ep.tile([128, NQ], BF16, tag="e", name=f"e{pair}_{c}_{i}")
                        nc.scalar.activation(
                            out=e_t, in_=ps_pair[i], func=AF.Exp, scale=SCALE
                        )
                        for nb in range(NQ // 512):
                            nc.tensor.matmul(
                                po_pair[i][:, 512 * nb : 512 * (nb + 1)],
                                (vt_sb[c][:, VTW * h : VTW * h + HD + 1]),
                                e_t[:, 512 * nb : 512 * (nb + 1)],
                                start=(c == 0),
                                stop=(c == NCHUNK - 1),
                            )
                for i in range(2):
                    head_epilogue(2 * pair + i, po_pair[i])

            # ---- tail: normalize heads, out-projection ------------------
            for h in range(NH):
                pb = pp.tile([64, NQ], F32, tag="pp", name=f"pb{h}")
                for nb in range(NQ // 512):
                    nc.tensor.matmul(
                        pb[:, 512 * nb : 512 * (nb + 1)],
                        ones64_sb,
                        rs_l[h][:, 512 * nb : 512 * (nb + 1)],
                        start=True,
                        stop=True,
                    )
                nc.vector.tensor_tensor(
                    out=attn_sb[h], in0=oh_l[h][0:64, :], in1=pb, op=mybir.AluOpType.mult
                )
                for oct_ in range(2):
                    pf = po.tile([128, NQ], F32, tag="po", name=f"pf{h}_{oct_}")
                    for nb in range(NQ // 512):
                        nc.tensor.matmul(
                            pf[:, 512 * nb : 512 * (nb + 1)],
                            (wo_sb[h][:, 128 * oct_ : 128 * (oct_ + 1)]),
                            attn_sb[h][:, 512 * nb : 512 * (nb + 1)],
                            start=True,
                            stop=True,
                        )
                    if h == 0:
                        nc.vector.tensor_copy(out=oacc_sb[oct_], in_=pf)
                    else:
                        nc.vector.tensor_tensor(
                            out=oacc_sb[oct_],
                            in0=oacc_sb[oct_],
                            in1=pf,
                            op=mybir.AluOpType.add,
                        )

        # ---- bias + residual + store -----------------------------------
        with tc.tile_pool(name="fpool", bufs=2) as fp:
            for oct_ in range(2):
                fo = fp.tile([128, NQ], F32, tag="fo")
                nc.scalar.activation(
                    out=fo, in_=oacc_sb[oct_], func=AF.Identity, bias=ob_sb[oct_]
                )
                nc.vector.tensor_tensor(
                    out=fo, in0=fo, in1=x_sb[oct_][:, 0:NQ], op=mybir.AluOpType.add
                )
                nc.sync.dma_start(out=y_d[128 * oct_ : 128 * (oct_ + 1), :], in_=fo)


_PROGRAM = None


def _get_program():
    global _PROGRAM
    if _PROGRAM is None:
        _PROGRAM = _build_program()
    return _PROGRAM


def _prep_inputs(input, gn_weight, gn_bias, qkv_weight, out_weight, out_bias):
    input = np.asarray(input, dtype=np.float32).reshape(B, C, S)
    gn_weight = np.asarray(gn_weight, dtype=np.float32)
    gn_bias = np.asarray(gn_bias, dtype=np.float32)
    qkv_weight = np.asarray(qkv_weight, dtype=np.float32)
    out_weight = np.asarray(out_weight, dtype=np.float32)
    out_bias = np.asarray(out_bias, dtype=np.float32)

    # reference splits qkv head-major: rows 192h..192h+192 = [q|k|v] of head h.
    # device layout wants cols [Q heads 0..3 | K heads 0..3 | V heads 0..3].
    perm = np.concatenate(
        [
            np.arange(192 * h + 64 * part, 192 * h + 64 * (part + 1))
            for part in range(3)
            for h in range(NH)
        ]
    )
    import ml_dtypes

    wqkvT = np.ascontiguousarray(qkv_weight.T[:, perm]).astype(ml_dtypes.bfloat16)
    # reorder: per head h, q rows h*64..h*64+64 map to wqkvT cols as-is
    woutT = np.ascontiguousarray(out_weight.T.reshape(NH, HD, C)).astype(
        ml_dtypes.bfloat16
    )
    gnw = np.ascontiguousarray(gn_weight.reshape(2, 128, 1))
    gnb = np.ascontiguousarray(gn_bias.reshape(2, 128, 1))
    ob = np.ascontiguousarray(out_bias.reshape(2, 128, 1))
    gsel = np.zeros((128, 8), np.float32)
    for p in range(128):
        gsel[p, p // 16] = 1.0
    gselT = np.ascontiguousarray(gsel.T)

    in_maps = []
    for core in range(8):
        b, j = core // 4, core % 4
        xrot = np.roll(input[b], -NQ * j, axis=1)
        in_maps.append(
            {
                "x": np.ascontiguousarray(xrot),
                "wqkvT": wqkvT,
                "woutT": woutT,
                "gnw": gnw,
                "gnb": gnb,
                "ob": ob,
                "gsel": gsel,
                "gselT": gselT,
            }
        )
    return in_maps


def kernel(input, gn_weight, gn_bias, qkv_weight, out_weight, out_bias, _trace=False):
    nc = _get_program()
    in_maps = _prep_inputs(
        input, gn_weight, gn_bias, qkv_weight, out_weight, out_bias
    )
    kw = {}
    if _trace:
        kw = {"trace": True, "tmpdir": "/tmp/attn_trace"}
    res = run_bass_kernel_spmd(nc, in_maps, list(range(8)), **kw)
    out = np.empty((B, C, S), np.float32)
    for core in range(8):
        b, j = core // 4, core % 4
        out[b, :, NQ * j : NQ * (j + 1)] = res.results[core]["y"]
    out = out.reshape(B, C, H, W)
    if _trace:
        return out, res
    return out
